# revision 22
# baseline (speedup 1.0000x reference)
"""Trainium2 Bass kernel for batch-axis-softmax dot-product attention.

Problem: B=8, S=4096, D=64 fp32.
    scores = einsum('bqd,bkd->bqk', Q, K) / 8
    attn   = softmax(scores, axis=0)          # over the BATCH axis!
    out    = einsum('bqk,bkd->bqd', attn, V)

The batch-axis softmax couples only the 8 batch entries of a fixed (q, k)
position, so sharding over the *query* axis (512 queries per core, K/V
replicated) keeps the softmax fully local to each core.

Design (HW-measured journey: 198us baseline -> ~170us): the kernel is
jointly ScalarE- and VectorE-bound (~145us of exp on ACT, ~142us of
adds/mults/recip on DVE; PE ~115us), so every change targets one of those
queues:
  * The softmax reciprocal left ScalarE entirely: a runtime-registered
    custom DVE op RECIP_SUM_ANT computes 1/(a+b) (BITWISE_NOT exponent
    seed + 1 Newton pass, ~0.2% max err) fusing the last batch-tree add
    with the reciprocal. ScalarE runs ONLY the 128 exp ops.
  * DVE work is batched per QUAD of k-tiles into 4 wide ops (tree L1/L2
    fused 3D-AP adds, RECIP_SUM, one broadcast multiply done IN-PLACE
    over the exp tile) to amortize the ~160cy per-op dispatch overhead;
    the in-place multiply frees the SBUF a separate W pool would need.
  * Pipeline: exps(g) land in window g, tree+recip(g) in g+1, mult+AVs(g)
    in g+2. The first/last quads run per-tile (the very first tile
    pack-wise) so the DVE queue starts as early and drains as late-shifted
    as the data allows; the last two quads' back-ends are emitted before
    the final front end to avoid head-of-line blocking ready DVE work
    behind the final exps.

Per-core pipeline, per k-tile QUAD (4 x 128 keys x 512 queries, 8 batches):
  PE : scoresT[k,q] = K_tile @ Q^T per tile (fp16 in, fp32 psum; batch pairs
       row-packed via tile_position) -> 16 psum packs [128,1024] per quad
  ACT: e16[128, 16384] = exp(0.125 * scores) (16 ops, fp16 SBUF)
  DVE: t16 = L1 add, v16 = L2 add, r16 = RECIP_SUM(v16 halves),
       e16 *= r16-broadcast in place (fp16 2x mode except recip at 1x)
  PE : outT_b[d,q] += V_tile matmul per tile, accumulated across k in
       persistent psum (2 batches per bank via column tiling; start=True
       k-tile MUST execute first in each bank - has_written clear)
Epilogue: psum -> fp16 sbuf via 2 DVE + 2 ACT copies (both engines idle by
then), per-pair output DMAs; host converts fp16 -> fp32.
"""

import numpy as np

B = 8
S = 4096
D = 64
NCORES = 8
QBLK = S // NCORES  # 512 queries per core
KT = 128            # keys per k-tile
NKT = S // KT       # 32 k-tiles
NPAIR = B // 2      # batch pairs packed into 128 partitions
QD = 4              # k-tiles per DVE work group (quad)
NQ = NKT // QD      # 8 quads

# test.py can flip these before calling kernel()
TRACE = False
TRACE_KWARGS = {}
LAST_RESULT = None  # BassKernelResults of the most recent run (for profiling)

_cache = {}

# Chebyshev seed constants shared with RECIPROCAL_APPROX_FAST (dve_ops.py).
RECIP_SUM_CONSTS = {"s0": -0.23549792, "s1": 2.0017324}


def _register_recip_sum():
    """Register a custom DVE op RECIP_SUM_ANT: out = approx 1/(in0 + in1)
    (BITWISE_NOT exponent-flip seed + one inline Newton-Raphson pass,
    ~0.2% max rel err). Fuses the final batch-tree add with the softmax
    denominator reciprocal into one 1x-rate DVE instruction."""
    import numpy as np  # noqa: F811

    from concourse import dve_ops
    from concourse.dve_spec import AluOp, Bin, C0, C1, Spec, Src0, Src1, _has_src1, lower
    from concourse.dve_uop import DveOpSpec

    NAME = "RECIP_SUM_ANT"
    for op in dve_ops.OPS:
        if op.name == NAME:
            return op

    s = Src0 + Src1
    ns = Bin(AluOp.BITWISE_NOT, s, s)
    y0 = ns * C0
    y1 = y0 * (C1 - s * y0)

    def ref(in0, in1, c0, c1, c2):
        z = (in0.astype(np.float32) + in1.astype(np.float32)).astype(np.float32)
        not_x = (~np.ascontiguousarray(z).view(np.int32)).view(np.float32)
        yy0 = not_x * c0
        return yy0 * (c1 - z * yy0)

    spec = Spec(body=y1, reference=ref)
    row = dve_ops._CUSTOM_DVE_ROW_BASE + len(dve_ops.OPS)
    assert row < 0x20
    shas = {}
    for ver in ("v3", "v4"):
        try:
            compiled = DveOpSpec(
                name=NAME, opcode=row, uops=lower(spec, ver=ver), rd1_en=_has_src1(spec)
            )
            shas[ver] = compiled.sha(ver)
        except Exception:
            pass  # only the current arch's ver is required
    op = dve_ops.DveOp(NAME, spec, subdim=False, uops_sha=shas)
    dve_ops.OPS.append(op)
    dve_ops.CUSTOM_DVE_SPECS[NAME] = spec
    dve_ops._SUB_OPCODE_FOR_NAME[NAME] = row
    return op


def _build_nc():
    from contextlib import ExitStack

    import concourse.tile as tile
    from concourse import bacc, mybir

    f16 = mybir.dt.float16
    f32 = mybir.dt.float32
    Exp = mybir.ActivationFunctionType.Exp
    recip_sum_op = _register_recip_sum()

    # Bacc (not raw Bass): its finalize() runs the legalization passes that
    # split multi-wait sync_info into EventSemaphore instructions (TRN2 allows
    # at most one wait per regular instruction).
    nc = bacc.Bacc()

    # Inputs pre-arranged on host into exact SBUF layouts (fp16):
    #   qt[p, j*512 + q] = Q[2j + p//64, cblk*512 + q, p%64]
    #   kt[p, j*4096 + k] = K[2j + p//64, k, p%64]
    #   vv[p, b*2048 + n*64 + d] = V[b, n*128 + p, d]
    qt_d = nc.dram_tensor("qt", [128, NPAIR * QBLK], f16, kind="ExternalInput")
    kt_d = nc.dram_tensor("kt", [128, NPAIR * S], f16, kind="ExternalInput")
    vv_d = nc.dram_tensor("vv", [128, B * NKT * D], f16, kind="ExternalInput")
    # out[j][(b%2)*64 + d, q] = out_bqd[2j + b%2, q, d], fp16 (host widens)
    out_d = nc.dram_tensor("out", [NPAIR, 128, QBLK], f16, kind="ExternalOutput")

    RC = RECIP_SUM_CONSTS

    with tile.TileContext(nc) as tc, ExitStack() as ctx:
        in_p = ctx.enter_context(tc.tile_pool(name="inp", bufs=1))
        e_p = ctx.enter_context(tc.tile_pool(name="e16", bufs=3))
        t_p = ctx.enter_context(tc.tile_pool(name="t16", bufs=1))
        v_p = ctx.enter_context(tc.tile_pool(name="v16", bufs=1))
        r_p = ctx.enter_context(tc.tile_pool(name="r16", bufs=2))
        st_p = ctx.enter_context(tc.tile_pool(name="stage", bufs=1))
        ps_s = ctx.enter_context(tc.tile_pool(name="ps_s", bufs=2, space="PSUM"))
        ps_o = ctx.enter_context(tc.tile_pool(name="ps_o", bufs=1, space="PSUM"))

        # kt/vv are laid out k-tile-major on the host and DMA'd in per-tile
        # chunks interleaved kt/vv, so tile 0's operands land ~7us in and the
        # loop never waits on later chunks.
        qt = in_p.tile([128, NPAIR * QBLK], f16)
        kt = in_p.tile([128, NKT * NPAIR * KT], f16)
        vv = in_p.tile([128, NKT * B * D], f16)
        CH = NPAIR * KT  # 512 columns per k-tile chunk (for both kt and vv)

        def dma_col(dst, src, c0, c1):
            nc.sync.dma_start(out=dst[:, c0:c1], in_=src[:, c0:c1])

        # Issue order: operands of score pack (t=0, j=0) first, then the
        # rest of tile 0, then per-tile chunks so the loop never waits.
        dma_col(qt, qt_d, 0, QBLK)
        dma_col(kt, kt_d, 0, KT)       # 32KB: pack (t=0, j=0) unblocks early
        dma_col(kt, kt_d, KT, CH)
        for j in range(1, NPAIR):
            dma_col(qt, qt_d, j * QBLK, (j + 1) * QBLK)
        dma_col(vv, vv_d, 0, CH)
        for t in range(1, NKT):
            dma_col(kt, kt_d, t * CH, (t + 1) * CH)
            dma_col(vv, vv_d, t * CH, (t + 1) * CH)

        # Persistent output accumulators: bank j holds batches 2j (parts
        # 0:64) and 2j+1 (parts 64:128), accumulated over all 32 k-tiles.
        oacc = [
            ps_o.tile([128, QBLK], f32, tag=f"oacc{j}", name=f"oacc{j}")
            for j in range(NPAIR)
        ]

        # AV matmuls pending issue; drained between score packs so PE always
        # services the (ACT-feeding) score matmuls promptly instead of
        # running long AV bursts that starve ScalarE. Interleaving AV MMs
        # of different k-tiles is safe: psum accumulate-adds commute.
        av_pending = []

        def drain_av(n):
            for _ in range(min(n, len(av_pending))):
                av_pending.pop(0)()

        def emit_scores_exp(g, e16, taus):
            # scores + exp for tiles `taus` of quad g; one 2-bank psum pack
            # per (tile, batch-pair), exp'd into the shared e16 tile.
            for tau in taus:
                u = QD * g + tau
                for j in range(NPAIR):
                    sc = ps_s.tile([128, 2 * QBLK], f32, tag="sc", name=f"sc{u}_{j}")
                    for m in range(2):  # m=0 -> b=2j (rows 0:64), m=1 -> b=2j+1
                        rb = m * 64
                        nc.tensor.matmul(
                            out=sc[:, m * QBLK : (m + 1) * QBLK],
                            lhsT=kt[rb : rb + 64, u * CH + j * KT : u * CH + (j + 1) * KT],
                            rhs=qt[rb : rb + 64, j * QBLK : (j + 1) * QBLK],
                            start=True,
                            stop=True,
                            tile_position=(rb, 0),
                        )
                    # E = exp(scores / sqrt(D)); scores*0.125 in [-6, 6] so no
                    # max-subtraction is needed and fp16 range is safe.
                    off = tau * 4096 + j * 1024
                    nc.scalar.activation(e16[:, off : off + 1024], sc[:], Exp, scale=0.125)
                    drain_av(2)

        def tree_tile(e16, tau, t16, v16, r16, packwise=False):
            # Per-tile tree chain (3 ops) for head/tail quads.
            eb = tau * 4096
            tb = tau * 2048
            if packwise:
                # Pack-level first level: a needs only exps 0-1, b only 2-3,
                # so the DVE pipeline starts two exps (~2.3us) earlier at the
                # kernel head. (a|b) = (E0+E2|E1+E3), (E4+E6|E5+E7); summing
                # halves later still yields the full 8-batch Z.
                nc.vector.tensor_add(
                    t16[:, tb : tb + 1024], e16[:, eb : eb + 1024], e16[:, eb + 1024 : eb + 2048]
                )
                nc.vector.tensor_add(
                    t16[:, tb + 1024 : tb + 2048],
                    e16[:, eb + 2048 : eb + 3072],
                    e16[:, eb + 3072 : eb + 4096],
                )
            else:
                nc.vector.tensor_add(
                    t16[:, tb : tb + 2048],
                    e16[:, eb : eb + 2048],
                    e16[:, eb + 2048 : eb + 4096],
                )
            nc.vector.tensor_add(
                v16[:, tau * 1024 : (tau + 1) * 1024],
                t16[:, tb : tb + 1024],
                t16[:, tb + 1024 : tb + 2048],
            )
            nc.vector._custom_dve(
                recip_sum_op,
                out=r16[:, tau * 512 : (tau + 1) * 512],
                in0=v16[:, tau * 1024 : tau * 1024 + 512],
                in1=v16[:, tau * 1024 + 512 : (tau + 1) * 1024],
                s0=RC["s0"],
                s1=RC["s1"],
            )

        def emit_tree_recip(g, e16, fine=False):
            # r16[(tau,q) cols] = approx 1/(sum_b E_b). Two fused 3D-AP adds
            # (L1/L2, fp16 2x mode) spanning all 4 tiles, then ONE custom-DVE
            # op that both sums the last pair of partials and takes the
            # reciprocal (fp16 in-pipe converts to fp32 before the
            # BITWISE_NOT so the fp32-bit-layout seed still holds).
            # fine=True runs the chain per TILE so the first quad doesn't
            # serialize behind the full 16-exp barrier.
            t16 = t_p.tile([128, QD * 2048], f16, tag="t16", name=f"t16_{g}")
            v16 = v_p.tile([128, QD * 1024], f16, tag="v16", name=f"v16_{g}")
            r16 = r_p.tile([128, QD * 512], f16, tag="r16", name=f"r16_{g}")
            if fine:
                for tau in range(QD):
                    tree_tile(e16, tau, t16, v16, r16, packwise=(g == 0 and tau == 0))
                return r16
            e4 = e16[:].rearrange("p (t c) -> p t c", t=QD)
            t4 = t16[:].rearrange("p (t c) -> p t c", t=QD)
            nc.vector.tensor_add(t4, e4[:, :, 0:2048], e4[:, :, 2048:4096])
            v4 = v16[:].rearrange("p (t c) -> p t c", t=QD)
            nc.vector.tensor_add(v4, t4[:, :, 0:1024], t4[:, :, 1024:2048])
            r4 = r16[:].rearrange("p (t c) -> p t c", t=QD)
            nc.vector._custom_dve(
                recip_sum_op,
                out=r4,
                in0=v4[:, :, 0:512],
                in1=v4[:, :, 512:1024],
                s0=RC["s0"],
                s1=RC["s1"],
            )
            return r16

        def emit_mult(g, e16, r16):
            # W = E * (1/Z) IN PLACE over e16, r broadcast over the 8
            # batches via a stride-0 AP dim; fp16 2x mode, one op per quad.
            ew = e16[:].rearrange("p (t b q) -> p t b q", t=QD, b=8)
            rw = r16[:].rearrange("p (t a q) -> p t a q", t=QD, a=1)
            nc.vector.tensor_mul(ew, ew, rw.to_broadcast((128, QD, 8, 512)))

        def emit_mult_tile(g, e16, r16, tau):
            # Per-tile variant for the tail so each tile's AVs chase its mult.
            c0 = tau * 4096
            ew = e16[:, c0 : c0 + 4096].rearrange("p (b q) -> p b q", b=8)
            rw = r16[:, tau * 512 : (tau + 1) * 512].rearrange("p (a q) -> p a q", a=1)
            nc.vector.tensor_mul(ew, ew, rw.to_broadcast((128, 8, 512)))

        def emit_av_tile(g, e16, tau):
            # outT_b[d,q] += V_b[t]^T-form matmul, queued for interleaved
            # issue (see drain_av). e16 holds W after the in-place mult.
            # tau MUST be queued in forward order across calls: tile u=0's
            # start=True AV has to execute before any accumulate-AV or the
            # has_written clear wipes them. b order within a tile is free
            # (disjoint psum regions); the final tile goes b-ascending so the
            # per-pair epilogue copy+DMA for low j starts while high-j AVs
            # still run.
            u = QD * g + tau

            def mk(b):
                j, m = b // 2, b % 2
                rb = m * 64

                def go():
                    nc.tensor.matmul(
                        out=oacc[j][rb : rb + 64, :],
                        lhsT=vv[:, u * CH + b * D : u * CH + (b + 1) * D],
                        rhs=e16[:, tau * 4096 + b * 512 : tau * 4096 + (b + 1) * 512],
                        start=(u == 0),
                        stop=(u == NKT - 1),
                        tile_position=(0, rb),
                        skip_group_check=True,
                    )

                return go

            bs = range(B) if u == NKT - 1 else reversed(range(B))
            for b in bs:
                av_pending.append(mk(b))

        # Software pipeline over k-tile quads: exps(g) land in window g,
        # tree+recip(g) run in window g+1, mult(g)+AVs(g) in window g+2, so
        # every cross-engine wait reaching an engine's strict FIFO was
        # produced >= 1 window earlier and is pre-satisfied.
        state = {}  # g -> (e16, r16)

        def back_end(gb, split):
            e16b, r16b = state.pop(gb)
            if not split:
                emit_mult(gb, e16b, r16b)
                for tau in range(QD):
                    emit_av_tile(gb, e16b, tau)
            else:
                # Per-tile mults so each tile's AVs chase its mult instead
                # of waiting for the full-quad op.
                for tau in range(QD):
                    emit_mult_tile(gb, e16b, r16b, tau)
                    emit_av_tile(gb, e16b, tau)
                    drain_av(16)

        for g in range(NQ - 1):
            if g >= 2:
                back_end(g - 2, split=False)
            e16 = e_p.tile([128, QD * 4096], f16, tag="e16", name=f"e16_{g}")
            emit_scores_exp(g, e16, taus=range(QD))
            # Quad 0 runs the tree per tile (tile 0 pack-wise) so the DVE
            # pipeline starts as soon as the first two exps land.
            r16 = emit_tree_recip(g, e16, fine=(g == 0))
            state[g] = (e16, r16)

        # End game. Pull the two outstanding back ends in BEFORE the final
        # quad's front end: their deps are already satisfied, and emitting
        # them after tree(last) would head-of-line block ready DVE work
        # behind the final exps. The last quad runs per-tile
        # exp->tree->mult->AV chains so only tile 31's short chain trails
        # the last exp of the kernel.
        gl = NQ - 1
        back_end(NQ - 3, split=False)
        back_end(NQ - 2, split=True)
        e16 = e_p.tile([128, QD * 4096], f16, tag="e16", name=f"e16_{gl}")
        t16 = t_p.tile([128, QD * 2048], f16, tag="t16", name=f"t16_{gl}")
        v16 = v_p.tile([128, QD * 1024], f16, tag="v16", name=f"v16_{gl}")
        r16 = r_p.tile([128, QD * 512], f16, tag="r16", name=f"r16_{gl}")
        for tau in range(QD):
            emit_scores_exp(gl, e16, taus=(tau,))
            tree_tile(e16, tau, t16, v16, r16)
            emit_mult_tile(gl, e16, r16, tau)
            emit_av_tile(gl, e16, tau)
            drain_av(16)
        drain_av(len(av_pending))

        # Epilogue: psum -> fp16 staging split 2 DVE / 2 ACT (both engines
        # are idle once the final AVs drain); per-pair output DMAs overlap
        # the remaining AVs.
        st = st_p.tile([128, NPAIR * QBLK], f16, tag="st")
        for j in range(NPAIR):
            dst = st[:, j * QBLK : (j + 1) * QBLK]
            if j % 2 == 0:
                nc.scalar.copy(out=dst, in_=oacc[j][:])
            else:
                nc.vector.tensor_copy(out=dst, in_=oacc[j][:])
            nc.sync.dma_start(out=out_d[j, :, :], in_=dst)

    return nc


def _get_nc():
    if "nc" not in _cache:
        nc = _build_nc()
        if not nc.is_finalized():
            # Runs Bacc.compile() legalization (wait splitting, reg alloc).
            nc.finalize()
        _cache["nc"] = nc
    return _cache["nc"]


def _host_prep(queries, keys, values):
    """Cast to fp16 and pre-arrange into the SBUF layouts (see _build_nc)."""
    k16 = np.asarray(keys, dtype=np.float16)
    v16 = np.asarray(values, dtype=np.float16)
    q16 = np.asarray(queries, dtype=np.float16)

    # kt[(b%2)*64+d, t*512 + (b//2)*128 + kk] = K[b, t*128+kk, d] (k-tile major)
    kt = np.ascontiguousarray(
        k16.reshape(NPAIR, 2, NKT, KT, D)
        .transpose(1, 4, 2, 0, 3)
        .reshape(128, NKT * NPAIR * KT)
    )
    # vv[p, t*512 + b*64 + d] = V[b, t*128+p, d] (k-tile major)
    vv = np.ascontiguousarray(
        v16.reshape(B, NKT, KT, D).transpose(2, 1, 0, 3).reshape(128, NKT * B * D)
    )

    qts = []
    for c in range(NCORES):
        qc = q16[:, c * QBLK : (c + 1) * QBLK, :]  # [8, 512, 64]
        qt = np.ascontiguousarray(
            qc.transpose(0, 2, 1).reshape(NPAIR, 128, QBLK).transpose(1, 0, 2).reshape(128, NPAIR * QBLK)
        )
        qts.append(qt)
    return qts, kt, vv


def kernel(queries, keys, values):
    global LAST_RESULT
    from concourse.bass_utils import run_bass_kernel_spmd

    queries = np.asarray(queries, dtype=np.float32)
    keys = np.asarray(keys, dtype=np.float32)
    values = np.asarray(values, dtype=np.float32)

    nc = _get_nc()
    qts, kt, vv = _host_prep(queries, keys, values)
    in_maps = [{"qt": qts[c], "kt": kt, "vv": vv} for c in range(NCORES)]

    res = run_bass_kernel_spmd(
        nc,
        in_maps,
        list(range(NCORES)),
        trace=TRACE,
        **TRACE_KWARGS,
    )
    LAST_RESULT = res

    out = np.empty((B, S, D), dtype=np.float32)
    for c in range(NCORES):
        o = res.results[c]["out"]  # [4, 128, 512] = [j, (b%2)*64+d, q] fp16
        out[:, c * QBLK : (c + 1) * QBLK, :] = (
            o.astype(np.float32).reshape(B, D, QBLK).transpose(0, 2, 1)
        )
    return out


# revision 25
# speedup vs baseline: 1.0184x; 1.0184x over previous
"""Trainium2 Bass kernel for batch-axis-softmax dot-product attention.

Problem: B=8, S=4096, D=64 fp32.
    scores = einsum('bqd,bkd->bqk', Q, K) / 8
    attn   = softmax(scores, axis=0)          # over the BATCH axis!
    out    = einsum('bqk,bkd->bqd', attn, V)

The batch-axis softmax couples only the 8 batch entries of a fixed (q, k)
position, so sharding over the *query* axis (512 queries per core, K/V
replicated) keeps the softmax fully local to each core.

Design (HW-measured journey: 198us baseline -> 175us): the kernel is
jointly ScalarE- and VectorE-bound (~145us of exp on ACT, ~150us of
adds/mults/recip on DVE; PE ~115us), so every change targets one of those
queues:
  * The softmax reciprocal left ScalarE entirely: a runtime-registered
    custom DVE op RECIP_SUM_ANT computes 1/(a+b) (BITWISE_NOT exponent
    seed + 1 Newton pass, ~0.2% max err) fusing the last batch-tree add
    with the reciprocal. ScalarE runs ONLY the 128 exp ops.
  * DVE work is batched per k-tile PAIR into 4 wide ops (tree L1/L2 fused
    3D-AP adds, RECIP_SUM, one broadcast multiply) to amortize the ~160cy
    per-op dispatch overhead.
  * Pipeline: exps(p) land in window p, tree+recip(p) in p+1, mult+AVs(p)
    in p+2. First/last pairs run per-tile (and the very first tile
    pack-wise) so the saturated DVE queue starts as early and drains as
    late-shifted as the data allows; the last two pairs' back-ends are
    emitted before the final front end to avoid head-of-line blocking
    ~9us of ready DVE work behind the final exps.

Per-core pipeline, per k-tile PAIR (2 x 128 keys x 512 queries, 8 batches):
  PE : scoresT[k,q] = K_tile @ Q^T per tile (fp16 in, fp32 psum; batch pairs
       row-packed via tile_position) -> 8 psum packs [128,1024] per pair
  ACT: e8[128, 8192] = exp(0.125 * scores) (8 ops, fp16 SBUF)
  DVE: t8 = L1 add, v8 = L2 add, r8 = RECIP_SUM(v8 halves),
       w8 = e8 * r8-broadcast (fp16 2x mode throughout except recip)
  PE : outT_b[d,q] += V_tile matmul per tile, accumulated across k in
       persistent psum (2 batches per bank via column tiling; start=True
       k-tile MUST execute first in each bank - has_written clear)
Epilogue: psum -> fp16 sbuf via 4 ScalarE copies (ACT idles after the last
exp while DVE finishes), per-pair output DMAs; host converts fp16 -> fp32.
"""

import numpy as np

B = 8
S = 4096
D = 64
NCORES = 8
QBLK = S // NCORES  # 512 queries per core
KT = 128            # keys per k-tile
NKT = S // KT       # 32 k-tiles
NPAIR = B // 2      # batch pairs packed into 128 partitions
NTP = NKT // 2      # 16 k-tile pairs

# test.py can flip these before calling kernel()
TRACE = False
TRACE_KWARGS = {}
LAST_RESULT = None  # BassKernelResults of the most recent run (for profiling)

_cache = {}

# Chebyshev seed constants shared with RECIPROCAL_APPROX_FAST (dve_ops.py).
RECIP_SUM_CONSTS = {"s0": -0.23549792, "s1": 2.0017324}


def _register_recip_sum():
    """Register a custom DVE op RECIP_SUM_ANT: out = approx 1/(in0 + in1)
    (BITWISE_NOT exponent-flip seed + one inline Newton-Raphson pass,
    ~0.2% max rel err). Fuses the final batch-tree add with the softmax
    denominator reciprocal into one 1x-rate DVE instruction."""
    import numpy as np  # noqa: F811

    from concourse import dve_ops
    from concourse.dve_spec import AluOp, Bin, C0, C1, Spec, Src0, Src1, _has_src1, lower
    from concourse.dve_uop import DveOpSpec

    NAME = "RECIP_SUM_ANT"
    for op in dve_ops.OPS:
        if op.name == NAME:
            return op

    s = Src0 + Src1
    ns = Bin(AluOp.BITWISE_NOT, s, s)
    y0 = ns * C0
    y1 = y0 * (C1 - s * y0)

    def ref(in0, in1, c0, c1, c2):
        z = (in0.astype(np.float32) + in1.astype(np.float32)).astype(np.float32)
        not_x = (~np.ascontiguousarray(z).view(np.int32)).view(np.float32)
        yy0 = not_x * c0
        return yy0 * (c1 - z * yy0)

    spec = Spec(body=y1, reference=ref)
    row = dve_ops._CUSTOM_DVE_ROW_BASE + len(dve_ops.OPS)
    assert row < 0x20
    shas = {}
    for ver in ("v3", "v4"):
        try:
            compiled = DveOpSpec(
                name=NAME, opcode=row, uops=lower(spec, ver=ver), rd1_en=_has_src1(spec)
            )
            shas[ver] = compiled.sha(ver)
        except Exception:
            pass  # only the current arch's ver is required
    op = dve_ops.DveOp(NAME, spec, subdim=False, uops_sha=shas)
    dve_ops.OPS.append(op)
    dve_ops.CUSTOM_DVE_SPECS[NAME] = spec
    dve_ops._SUB_OPCODE_FOR_NAME[NAME] = row
    return op


def _build_nc():
    from contextlib import ExitStack

    import concourse.tile as tile
    from concourse import bacc, mybir

    f16 = mybir.dt.float16
    f32 = mybir.dt.float32
    Exp = mybir.ActivationFunctionType.Exp
    recip_sum_op = _register_recip_sum()

    # Bacc (not raw Bass): its finalize() runs the legalization passes that
    # split multi-wait sync_info into EventSemaphore instructions (TRN2 allows
    # at most one wait per regular instruction).
    nc = bacc.Bacc()

    # Inputs pre-arranged on host into exact SBUF layouts (fp16):
    #   qt[p, j*512 + q] = Q[2j + p//64, cblk*512 + q, p%64]
    #   kt[p, j*4096 + k] = K[2j + p//64, k, p%64]
    #   vv[p, b*2048 + n*64 + d] = V[b, n*128 + p, d]
    qt_d = nc.dram_tensor("qt", [128, NPAIR * QBLK], f16, kind="ExternalInput")
    kt_d = nc.dram_tensor("kt", [128, NPAIR * S], f16, kind="ExternalInput")
    vv_d = nc.dram_tensor("vv", [128, B * NKT * D], f16, kind="ExternalInput")
    # out[j][(b%2)*64 + d, q] = out_bqd[2j + b%2, q, d], fp16 (host widens)
    out_d = nc.dram_tensor("out", [NPAIR, 128, QBLK], f16, kind="ExternalOutput")

    RC = RECIP_SUM_CONSTS

    with tile.TileContext(nc) as tc, ExitStack() as ctx:
        in_p = ctx.enter_context(tc.tile_pool(name="inp", bufs=1))
        e_p = ctx.enter_context(tc.tile_pool(name="e8", bufs=3))
        w_p = ctx.enter_context(tc.tile_pool(name="w8", bufs=2))
        t_p = ctx.enter_context(tc.tile_pool(name="t8", bufs=2))
        v_p = ctx.enter_context(tc.tile_pool(name="v8", bufs=2))
        r_p = ctx.enter_context(tc.tile_pool(name="r8", bufs=3))
        st_p = ctx.enter_context(tc.tile_pool(name="stage", bufs=1))
        ps_s = ctx.enter_context(tc.tile_pool(name="ps_s", bufs=2, space="PSUM"))
        ps_o = ctx.enter_context(tc.tile_pool(name="ps_o", bufs=1, space="PSUM"))

        # kt/vv are laid out k-tile-major on the host and DMA'd in per-tile
        # chunks interleaved kt/vv, so tile 0's operands land ~7us in and the
        # loop never waits on later chunks.
        qt = in_p.tile([128, NPAIR * QBLK], f16)
        kt = in_p.tile([128, NKT * NPAIR * KT], f16)
        vv = in_p.tile([128, NKT * B * D], f16)
        CH = NPAIR * KT  # 512 columns per k-tile chunk (for both kt and vv)

        def dma_col(dst, src, c0, c1):
            nc.sync.dma_start(out=dst[:, c0:c1], in_=src[:, c0:c1])

        # Issue order: operands of score pack (t=0, j=0) first, then the
        # rest of tile 0, then per-tile chunks so the loop never waits.
        dma_col(qt, qt_d, 0, QBLK)
        dma_col(kt, kt_d, 0, KT)       # 32KB: pack (t=0, j=0) unblocks early
        dma_col(kt, kt_d, KT, CH)
        for j in range(1, NPAIR):
            dma_col(qt, qt_d, j * QBLK, (j + 1) * QBLK)
        dma_col(vv, vv_d, 0, CH)
        for t in range(1, NKT):
            dma_col(kt, kt_d, t * CH, (t + 1) * CH)
            dma_col(vv, vv_d, t * CH, (t + 1) * CH)

        # Persistent output accumulators: bank j holds batches 2j (parts
        # 0:64) and 2j+1 (parts 64:128), accumulated over all 32 k-tiles.
        oacc = [
            ps_o.tile([128, QBLK], f32, tag=f"oacc{j}", name=f"oacc{j}")
            for j in range(NPAIR)
        ]

        # AV matmuls pending issue; drained between score packs so PE always
        # services the (ACT-feeding) score matmuls promptly instead of
        # running long AV bursts that starve ScalarE. Interleaving AV MMs
        # of adjacent k-tiles is safe: psum accumulate-adds commute.
        av_pending = []

        def drain_av(n):
            for _ in range(min(n, len(av_pending))):
                av_pending.pop(0)()

        def emit_scores_exp(p, e8, t2s=(0, 1)):
            # scores + exp for tiles t2s of pair p; one 2-bank psum pack
            # per (tile, batch-pair), exp'd into the shared e8 tile.
            for t2 in t2s:
                u = 2 * p + t2
                for j in range(NPAIR):
                    sc = ps_s.tile([128, 2 * QBLK], f32, tag="sc", name=f"sc{u}_{j}")
                    for m in range(2):  # m=0 -> b=2j (rows 0:64), m=1 -> b=2j+1
                        rb = m * 64
                        nc.tensor.matmul(
                            out=sc[:, m * QBLK : (m + 1) * QBLK],
                            lhsT=kt[rb : rb + 64, u * CH + j * KT : u * CH + (j + 1) * KT],
                            rhs=qt[rb : rb + 64, j * QBLK : (j + 1) * QBLK],
                            start=True,
                            stop=True,
                            tile_position=(rb, 0),
                        )
                    # E = exp(scores / sqrt(D)); scores*0.125 in [-6, 6] so no
                    # max-subtraction is needed and fp16 range is safe.
                    off = t2 * 4096 + j * 1024
                    nc.scalar.activation(e8[:, off : off + 1024], sc[:], Exp, scale=0.125)
                    drain_av(2)

        def tree_tile(e8, t2, t8, v8, r8, packwise=False):
            eb = t2 * 4096
            if packwise:
                # Pack-level first level: a needs only exps 0-1, b only 2-3,
                # so the DVE pipeline starts two exps (~2.3us) earlier at the
                # kernel head. (a|b) = (E0+E2|E1+E3), (E4+E6|E5+E7); summing
                # halves later still yields the full 8-batch Z.
                tb = t2 * 2048
                nc.vector.tensor_add(
                    t8[:, tb : tb + 1024], e8[:, eb : eb + 1024], e8[:, eb + 1024 : eb + 2048]
                )
                nc.vector.tensor_add(
                    t8[:, tb + 1024 : tb + 2048],
                    e8[:, eb + 2048 : eb + 3072],
                    e8[:, eb + 3072 : eb + 4096],
                )
            else:
                nc.vector.tensor_add(
                    t8[:, t2 * 2048 : (t2 + 1) * 2048],
                    e8[:, eb : eb + 2048],
                    e8[:, eb + 2048 : eb + 4096],
                )
            nc.vector.tensor_add(
                v8[:, t2 * 1024 : (t2 + 1) * 1024],
                t8[:, t2 * 2048 : t2 * 2048 + 1024],
                t8[:, t2 * 2048 + 1024 : (t2 + 1) * 2048],
            )
            nc.vector._custom_dve(
                recip_sum_op,
                out=r8[:, t2 * 512 : (t2 + 1) * 512],
                in0=v8[:, t2 * 1024 : t2 * 1024 + 512],
                in1=v8[:, t2 * 1024 + 512 : (t2 + 1) * 1024],
                s0=RC["s0"],
                s1=RC["s1"],
            )

        def emit_tree_recip(p, e8, fine=False):
            # r8[(t2,q) cols] = approx 1/(sum_b E_b). Two fused 3D-AP adds
            # (L1/L2, fp16 2x mode) then ONE custom-DVE op that both sums
            # the last pair of partials and takes the reciprocal (seed + one
            # Newton pass; fp16 in-pipe converts to fp32 before the
            # BITWISE_NOT so the fp32-bit-layout seed still holds).
            # fine=True runs the chain per TILE (2x the ops) so the first/
            # last pairs don't serialize behind the full 8-exp barrier.
            t8 = t_p.tile([128, 2 * 2048], f16, tag="t8", name=f"t8_{p}")
            v8 = v_p.tile([128, 2 * 1024], f16, tag="v8", name=f"v8_{p}")
            r8 = r_p.tile([128, 2 * 512], f16, tag="r8", name=f"r8_{p}")
            if fine:
                for t2 in range(2):
                    tree_tile(e8, t2, t8, v8, r8, packwise=(p == 0 and t2 == 0))
                return r8
            e3 = e8[:].rearrange("p (t c) -> p t c", t=2)
            t3 = t8[:].rearrange("p (t c) -> p t c", t=2)
            nc.vector.tensor_add(t3, e3[:, :, 0:2048], e3[:, :, 2048:4096])
            v3 = v8[:].rearrange("p (t c) -> p t c", t=2)
            nc.vector.tensor_add(v3, t3[:, :, 0:1024], t3[:, :, 1024:2048])
            r3 = r8[:].rearrange("p (t c) -> p t c", t=2)
            nc.vector._custom_dve(
                recip_sum_op,
                out=r3,
                in0=v3[:, :, 0:512],
                in1=v3[:, :, 512:1024],
                s0=RC["s0"],
                s1=RC["s1"],
            )
            return r8

        def emit_mult(p, e8, r8, w8):
            # w8 = E * (1/Z) with r broadcast over the 8 batches via a
            # stride-0 AP dim; fp16 2x mode, one op per pair.
            ew = e8[:].rearrange("p (t b q) -> p t b q", t=2, b=8)
            ww = w8[:].rearrange("p (t b q) -> p t b q", t=2, b=8)
            rw = r8[:].rearrange("p (t a q) -> p t a q", t=2, a=1)
            nc.vector.tensor_mul(ww, ew, rw.to_broadcast((128, 2, 8, 512)))

        def emit_mult_t2(p, e8, r8, t2, w8):
            # Per-tile variant for the tail so each tile's AVs chase its mult.
            c0 = t2 * 4096
            ew = e8[:, c0 : c0 + 4096].rearrange("p (b q) -> p b q", b=8)
            ww = w8[:, c0 : c0 + 4096].rearrange("p (b q) -> p b q", b=8)
            rw = r8[:, t2 * 512 : (t2 + 1) * 512].rearrange("p (a q) -> p a q", a=1)
            nc.vector.tensor_mul(ww, ew, rw.to_broadcast((128, 8, 512)))

        def emit_av_tile(p, w8, t2):
            # outT_b[d,q] += V_b[t]^T-form matmul, queued for interleaved
            # issue (see drain_av).
            # t2 MUST be queued in forward order across calls: tile u=2p's
            # start=True AV has to execute before u=2p+1's accumulate-AVs or
            # the has_written clear wipes them. b order within a tile is free
            # (disjoint psum regions); the final tile goes b-ascending so the
            # per-pair epilogue copy+DMA for low j starts while high-j AVs
            # still run.
            u = 2 * p + t2

            def mk(b):
                j, m = b // 2, b % 2
                rb = m * 64

                def go():
                    nc.tensor.matmul(
                        out=oacc[j][rb : rb + 64, :],
                        lhsT=vv[:, u * CH + b * D : u * CH + (b + 1) * D],
                        rhs=w8[:, t2 * 4096 + b * 512 : t2 * 4096 + (b + 1) * 512],
                        start=(u == 0),
                        stop=(u == NKT - 1),
                        tile_position=(0, rb),
                        skip_group_check=True,
                    )

                return go

            bs = range(B) if u == NKT - 1 else reversed(range(B))
            for b in bs:
                av_pending.append(mk(b))

        # Software pipeline over k-tile pairs with a 2-pair lag between the
        # exp front end and the mult/AV back end: exps(p) land in window p,
        # tree+recip(p) run in window p+1, mult(p)+AVs(p) in window p+2, so
        # every cross-engine wait reaching an engine's strict FIFO was
        # produced >= 1 window earlier and is pre-satisfied.
        state = {}  # p -> (e8, r8)

        def back_end(pb, split):
            e8b, r8b = state.pop(pb)
            w8 = w_p.tile([128, 8192], f16, tag="w8", name=f"w8_{pb}")
            # Per-tile mults (vs one full-pair op): each tile's 8 AVs are
            # released after ~2.3us instead of 4.4us, smoothing the PE's
            # AV/MM1 interleave (a full-pair op bursts 16 AVs at once and
            # the burst delays the score matmuls that feed ScalarE).
            for t2 in range(2):
                emit_mult_t2(pb, e8b, r8b, t2, w8)
                emit_av_tile(pb, w8, t2)
                if split:
                    drain_av(16)

        for p in range(NTP - 1):
            if p >= 2:
                back_end(p - 2, split=False)
            e8 = e_p.tile([128, 2 * 4096], f16, tag="e8", name=f"e8_{p}")
            emit_scores_exp(p, e8)
            # Pair 0 runs the tree per tile so the DVE pipeline starts two
            # exps earlier.
            r8 = emit_tree_recip(p, e8, fine=(p == 0))
            state[p] = (e8, r8)

        # End game. Pull the two outstanding back ends in BEFORE the final
        # pair's front end: their deps are already satisfied, and emitting
        # them after tree(last) would head-of-line block ~9us of DVE work
        # behind the final exps. The last pair itself runs per-tile
        # exp->tree->mult->AV chains so only tile 31's short chain trails
        # the last exp of the kernel.
        pl = NTP - 1
        back_end(NTP - 3, split=False)
        back_end(NTP - 2, split=True)
        e8 = e_p.tile([128, 2 * 4096], f16, tag="e8", name=f"e8_{pl}")
        t8 = t_p.tile([128, 2 * 2048], f16, tag="t8", name=f"t8_{pl}")
        v8 = v_p.tile([128, 2 * 1024], f16, tag="v8", name=f"v8_{pl}")
        r8 = r_p.tile([128, 2 * 512], f16, tag="r8", name=f"r8_{pl}")
        w8 = w_p.tile([128, 8192], f16, tag="w8", name=f"w8_{pl}")
        for t2 in range(2):
            emit_scores_exp(pl, e8, t2s=(t2,))
            # packwise: the first tree add only needs the tile's first two
            # exps, shortening the chain that trails the kernel's last exp.
            tree_tile(e8, t2, t8, v8, r8, packwise=True)
            emit_mult_t2(pl, e8, r8, t2, w8)
            emit_av_tile(pl, w8, t2)
            drain_av(16)
        drain_av(len(av_pending))

        # Epilogue: psum -> fp16 staging on ScalarE (idle after the last
        # exp, while the DVE finishes the tail mults); per-pair output DMAs
        # overlap the remaining AVs.
        st = st_p.tile([128, NPAIR * QBLK], f16, tag="st")
        for j in range(NPAIR):
            dst = st[:, j * QBLK : (j + 1) * QBLK]
            nc.scalar.copy(out=dst, in_=oacc[j][:])
            nc.sync.dma_start(out=out_d[j, :, :], in_=dst)

    return nc


def _get_nc():
    if "nc" not in _cache:
        nc = _build_nc()
        if not nc.is_finalized():
            # Runs Bacc.compile() legalization (wait splitting, reg alloc).
            nc.finalize()
        _cache["nc"] = nc
    return _cache["nc"]


def _host_prep(queries, keys, values):
    """Cast to fp16 and pre-arrange into the SBUF layouts (see _build_nc)."""
    k16 = np.asarray(keys, dtype=np.float16)
    v16 = np.asarray(values, dtype=np.float16)
    q16 = np.asarray(queries, dtype=np.float16)

    # kt[(b%2)*64+d, t*512 + (b//2)*128 + kk] = K[b, t*128+kk, d] (k-tile major)
    kt = np.ascontiguousarray(
        k16.reshape(NPAIR, 2, NKT, KT, D)
        .transpose(1, 4, 2, 0, 3)
        .reshape(128, NKT * NPAIR * KT)
    )
    # vv[p, t*512 + b*64 + d] = V[b, t*128+p, d] (k-tile major)
    vv = np.ascontiguousarray(
        v16.reshape(B, NKT, KT, D).transpose(2, 1, 0, 3).reshape(128, NKT * B * D)
    )

    qts = []
    for c in range(NCORES):
        qc = q16[:, c * QBLK : (c + 1) * QBLK, :]  # [8, 512, 64]
        qt = np.ascontiguousarray(
            qc.transpose(0, 2, 1).reshape(NPAIR, 128, QBLK).transpose(1, 0, 2).reshape(128, NPAIR * QBLK)
        )
        qts.append(qt)
    return qts, kt, vv


def kernel(queries, keys, values):
    global LAST_RESULT
    from concourse.bass_utils import run_bass_kernel_spmd

    queries = np.asarray(queries, dtype=np.float32)
    keys = np.asarray(keys, dtype=np.float32)
    values = np.asarray(values, dtype=np.float32)

    nc = _get_nc()
    qts, kt, vv = _host_prep(queries, keys, values)
    in_maps = [{"qt": qts[c], "kt": kt, "vv": vv} for c in range(NCORES)]

    res = run_bass_kernel_spmd(
        nc,
        in_maps,
        list(range(NCORES)),
        trace=TRACE,
        **TRACE_KWARGS,
    )
    LAST_RESULT = res

    out = np.empty((B, S, D), dtype=np.float32)
    for c in range(NCORES):
        o = res.results[c]["out"]  # [4, 128, 512] = [j, (b%2)*64+d, q] fp16
        out[:, c * QBLK : (c + 1) * QBLK, :] = (
            o.astype(np.float32).reshape(B, D, QBLK).transpose(0, 2, 1)
        )
    return out
